# revision 6
# baseline (speedup 1.0000x reference)
"""ConvAttnPool (CAML-style label-wise attention) Trainium2 kernel.

Strategy: shard the label dim Y across 8 NeuronCores. Y=8921 is padded to
9216 = 8 cores x 1152 labels (9 y-tiles of 128). Each core redundantly
computes embedding+conv+tanh -> h [B, L, D] (cheap), then for its own label
slab computes attention scores, softmax over L, the alpha slab output
[B, 1152, L] (the dominant ~92MB/core HBM write), and its logits/yhat/loss
partials.  No collectives: the host concatenates per-core outputs.

Key device tricks:
  - softmax denominator comes free from the Exp activation's accum_out
  - logits numerator sum_l exp(s)*c uses the fused DVE tensor_tensor_reduce
    (c[y,l] = final_w[y]·h[l] so m = alpha@h is never materialized and no
    transpose of alpha is needed)
  - normalization passes are split between ScalarE and VectorE to balance
    engine load; alpha tiles stream straight to HBM.
"""

import math
import numpy as np

B = 8
L = 2500
E = 100
D = 128
Y = 8921
K = 9
PADH = K // 2  # 4
VOCAB = 50002

NCORES = 8
NYT = 9                    # y-tiles of 128 per core
YPC = NYT * 128            # 1152 labels per core
LC = L // 5                # 500, psum chunk
NT = math.ceil(L / 128)    # 20 token tiles
L_PAD = NT * 128           # 2560

_CACHED_NC = [None]
RUN_KWARGS = {}            # test.py may set dict(trace=True, tmpdir=...)
LAST_RESULTS = [None]      # BassKernelResults from the last run


def _build_nc():
    import concourse.bass as bass
    import concourse.mybir as mybir
    from concourse import bacc
    from concourse.tile import TileContext
    from concourse.masks import make_identity

    f32 = mybir.dt.float32
    i32 = mybir.dt.int32
    AF = mybir.ActivationFunctionType
    OP = mybir.AluOpType

    nc = bacc.Bacc(None, target_bir_lowering=False)

    emb = nc.dram_tensor("emb", [VOCAB, 128], f32, kind="ExternalInput")
    xr = nc.dram_tensor("xr", [B, 128, NT], i32, kind="ExternalInput")
    cw = nc.dram_tensor("cw", [128, K, 128], f32, kind="ExternalInput")   # (e,k,d)
    cb = nc.dram_tensor("cb", [128, 1], f32, kind="ExternalInput")
    ut = nc.dram_tensor("ut", [128, YPC], f32, kind="ExternalInput")      # U^T slab
    fwt = nc.dram_tensor("fwt", [128, YPC], f32, kind="ExternalInput")    # final_w^T slab
    fbr = nc.dram_tensor("fbr", [128, NYT], f32, kind="ExternalInput")    # final_b
    mkr = nc.dram_tensor("mkr", [128, NYT], f32, kind="ExternalInput")    # validity mask
    tgt = nc.dram_tensor("tgt", [128, NYT * B], f32, kind="ExternalInput")

    alpha = nc.dram_tensor("alpha", [B, YPC, L], f32, kind="ExternalOutput")
    yhat = nc.dram_tensor("yhat", [128, NYT * B], f32, kind="ExternalOutput")
    lossp = nc.dram_tensor("lossp", [128, 1], f32, kind="ExternalOutput")

    with TileContext(nc) as tc:
        with (
            tc.tile_pool(name="const", bufs=1) as const,
            tc.tile_pool(name="ebig", bufs=1) as ebig,
            tc.tile_pool(name="etp", bufs=2) as etp,
            tc.tile_pool(name="hpool", bufs=1) as hpool,
            tc.tile_pool(name="xp", bufs=2) as xp,
            tc.tile_pool(name="expp", bufs=3) as expp,
            tc.tile_pool(name="alp", bufs=3) as alp,
            tc.tile_pool(name="small", bufs=1) as small,
            tc.tile_pool(name="recp", bufs=3) as recp,
            tc.tile_pool(name="trashp", bufs=1) as trashp,
            tc.tile_pool(name="ps_s", bufs=1, space="PSUM") as ps_s,
            tc.tile_pool(name="ps_e", bufs=1, space="PSUM") as ps_e,
            tc.tile_pool(name="ps_c", bufs=2, space="PSUM") as ps_c,
        ):
            identity = const.tile([128, 128], f32)
            make_identity(nc, identity[:])
            ut_sb = const.tile([128, YPC], f32)
            nc.sync.dma_start(ut_sb[:], ut[:])
            fwt_sb = const.tile([128, YPC], f32)
            nc.sync.dma_start(fwt_sb[:], fwt[:])
            cw_sb = const.tile([128, K, 128], f32)
            nc.sync.dma_start(cw_sb[:], cw[:])
            cb_sb = const.tile([128, 1], f32)
            nc.sync.dma_start(cb_sb[:], cb[:])
            fb_sb = const.tile([128, NYT], f32)
            nc.sync.dma_start(fb_sb[:], fbr[:])
            mk_sb = const.tile([128, NYT], f32)
            nc.sync.dma_start(mk_sb[:], mkr[:])
            tgt_sb = const.tile([128, NYT * B], f32)
            nc.sync.dma_start(tgt_sb[:], tgt[:])

            num_all = small.tile([128, NYT * B], f32)
            den_all = small.tile([128, NYT * B], f32)
            hT = hpool.tile([128, B, L], f32)

            # ---------------- Phase A: embedding + conv + tanh -> hT ------------
            for b in range(B):
                xb = xp.tile([128, NT], i32)
                nc.sync.dma_start(xb[:], xr[b])
                eb = ebig.tile([128, NT, 128], f32)
                for t in range(NT):
                    # NB: a single gather with a [128, NT] offset AP returns
                    # garbage on HW (works in sim) — keep one offset column per op.
                    nc.gpsimd.indirect_dma_start(
                        out=eb[:, t, :],
                        out_offset=None,
                        in_=emb[:],
                        in_offset=bass.IndirectOffsetOnAxis(ap=xb[:, t:t + 1], axis=0),
                    )
                etb = etp.tile([128, 2 * PADH + L], f32)
                nc.vector.memset(etb[:, 0:PADH], 0.0)
                nc.vector.memset(etb[:, PADH + L: 2 * PADH + L], 0.0)
                for g in range(math.ceil(NT / 4)):
                    eps = ps_e.tile([128, 512], f32, tag="eps")
                    for j in range(min(4, NT - g * 4)):
                        t = g * 4 + j
                        nc.tensor.transpose(
                            eps[:, j * 128:(j + 1) * 128], eb[:, t, :], identity[:]
                        )
                    ncols = min(512, L - g * 512)
                    nc.scalar.copy(
                        etb[:, PADH + g * 512: PADH + g * 512 + ncols], eps[:, :ncols]
                    )
                sps = ps_s.tile([128, 5, 512], f32, tag="sps")
                for k in range(K):
                    for c in range(5):
                        nc.tensor.matmul(
                            sps[:, c, :LC],
                            lhsT=cw_sb[:, k, :],
                            rhs=etb[:, k + c * LC: k + c * LC + LC],
                            start=(k == 0),
                            stop=(k == K - 1),
                        )
                nc.scalar.activation(
                    hT[:, b, :].rearrange("p (c l) -> p c l", c=5),
                    sps[:, :, :LC],
                    AF.Tanh,
                    bias=cb_sb[:, 0:1],
                )

            # ---------------- Phase B: per (y-tile, batch) ----------------------
            for yt in range(NYT):
                for b in range(B):
                    idx = yt * B + b
                    ysl = slice(yt * 128, (yt + 1) * 128)
                    sps = ps_s.tile([128, 5, 512], f32, tag="sps")
                    for c in range(5):
                        nc.tensor.matmul(
                            sps[:, c, :LC],
                            lhsT=ut_sb[:, ysl],
                            rhs=hT[:, b, c * LC:(c + 1) * LC],
                            start=True,
                            stop=True,
                        )
                    expt = expp.tile([128, L], f32)
                    nc.scalar.activation(
                        expt[:].rearrange("p (c l) -> p c l", c=5),
                        sps[:, :, :LC],
                        AF.Exp,
                        accum_out=den_all[:, idx:idx + 1],
                    )
                    acc5 = recp.tile([128, 5], f32, tag="acc5")
                    for c in range(5):
                        cps = ps_c.tile([128, 512], f32, tag="cps")
                        nc.tensor.matmul(
                            cps[:, :LC],
                            lhsT=fwt_sb[:, ysl],
                            rhs=hT[:, b, c * LC:(c + 1) * LC],
                            start=True,
                            stop=True,
                        )
                        trash = trashp.tile([128, LC], f32, tag="trash")
                        # fused multiply + per-partition sum on DVE.
                        # (tensor_tensor_reduce crashes on HW; op0=bypass breaks
                        # the accumulator — op0=mult with scalar 1.0 is exact.)
                        nc.vector.scalar_tensor_tensor(
                            out=trash[:],
                            in0=expt[:, c * LC:(c + 1) * LC],
                            scalar=1.0,
                            in1=cps[:, :LC],
                            op0=OP.mult,
                            op1=OP.mult,
                            accum_out=acc5[:, c:c + 1],
                        )
                    nc.vector.reduce_sum(
                        num_all[:, idx:idx + 1], acc5[:], axis=mybir.AxisListType.X
                    )
                    rec = recp.tile([128, 1], f32, tag="rec")
                    nc.vector.reciprocal(rec[:], den_all[:, idx:idx + 1])
                    alpt = alp.tile([128, L], f32)
                    if idx % 2 == 0:
                        nc.vector.tensor_scalar_mul(alpt[:], expt[:], rec[:])
                    else:
                        nc.scalar.activation(alpt[:], expt[:], AF.Copy, scale=rec[:, 0:1])
                    nc.sync.dma_start(alpha[b, ysl, :], alpt[:])

            # ---------------- Final: logits, yhat, masked BCE partials ----------
            NB = NYT * B
            recall = small.tile([128, NB], f32)
            nc.vector.reciprocal(recall[:], den_all[:])
            z = small.tile([128, NB], f32)
            nc.vector.tensor_tensor(z[:], num_all[:], recall[:], op=OP.mult)
            z3 = z[:].rearrange("p (t b) -> p t b", t=NYT)
            nc.vector.tensor_tensor(
                z3, z3, fb_sb[:, :, None].to_broadcast([128, NYT, B]), op=OP.add
            )
            yh = small.tile([128, NB], f32)
            nc.scalar.activation(yh[:], z[:], AF.Sigmoid)
            nc.sync.dma_start(yhat[:], yh[:])
            reluz = small.tile([128, NB], f32)
            nc.vector.tensor_scalar_max(reluz[:], z[:], 0.0)
            zt = small.tile([128, NB], f32)
            nc.vector.tensor_tensor(zt[:], z[:], tgt_sb[:], op=OP.mult)
            negabs = small.tile([128, NB], f32)
            nc.vector.scalar_tensor_tensor(
                out=negabs[:], in0=reluz[:], scalar=-2.0, in1=z[:],
                op0=OP.mult, op1=OP.add,
            )
            enb = small.tile([128, NB], f32)
            nc.scalar.activation(enb[:], negabs[:], AF.Exp)
            sp = small.tile([128, NB], f32)
            nc.scalar.activation(sp[:], enb[:], AF.Ln, bias=1.0)
            term = small.tile([128, NB], f32)
            nc.vector.tensor_tensor(term[:], reluz[:], zt[:], op=OP.subtract)
            nc.vector.tensor_tensor(term[:], term[:], sp[:], op=OP.add)
            t3 = term[:].rearrange("p (t b) -> p t b", t=NYT)
            nc.vector.tensor_tensor(
                t3, t3, mk_sb[:, :, None].to_broadcast([128, NYT, B]), op=OP.mult
            )
            lp = small.tile([128, 1], f32)
            nc.vector.reduce_sum(lp[:], term[:], axis=mybir.AxisListType.X)
            nc.sync.dma_start(lossp[:], lp[:])

    nc.finalize()
    return nc


def _get_nc():
    if _CACHED_NC[0] is None:
        _CACHED_NC[0] = _build_nc()
    return _CACHED_NC[0]


def _install_ntff_hook():
    """The RL container's antenv lacks axon_hooks; shim it so trace=True works."""
    import sys, types
    if "antenv.axon_hooks" in sys.modules:
        return
    m = types.ModuleType("antenv.axon_hooks")
    holder = [None]
    m.set_axon_ntff_profile_hook = lambda h: holder.__setitem__(0, h)
    m.get_axon_ntff_profile_hook = lambda: holder[0]
    sys.modules["antenv.axon_hooks"] = m
    try:
        import antenv
        antenv.axon_hooks = m
        from trn_agent_boot.trn_boot import _ntff_profile_via_ctypes
        m.set_axon_ntff_profile_hook(_ntff_profile_via_ctypes("/opt/axon/libaxon_pjrt.so"))
    except Exception:
        pass


def kernel(x, target, emb_table, conv_w, conv_b, U, final_w, final_b):
    from concourse.bass_utils import run_bass_kernel_spmd

    x = np.asarray(x).astype(np.int32)
    target = np.asarray(target, dtype=np.float32)
    emb_table = np.asarray(emb_table, dtype=np.float32)
    conv_w = np.asarray(conv_w, dtype=np.float32)
    conv_b = np.asarray(conv_b, dtype=np.float32)
    U = np.asarray(U, dtype=np.float32)
    final_w = np.asarray(final_w, dtype=np.float32)
    final_b = np.asarray(final_b, dtype=np.float32)

    # ---- host-side shard prep -------------------------------------------
    emb_pad = np.zeros((VOCAB, 128), dtype=np.float32)
    emb_pad[:, :E] = emb_table
    x_pad = np.zeros((B, L_PAD), dtype=np.int32)
    x_pad[:, :L] = x
    x_dev = np.ascontiguousarray(x_pad.reshape(B, NT, 128).transpose(0, 2, 1))
    cw_dev = np.zeros((128, K, 128), dtype=np.float32)
    cw_dev[:E] = conv_w.transpose(1, 2, 0)  # (e,k,d)
    cb_dev = np.ascontiguousarray(conv_b.reshape(128, 1))

    YT = NCORES * YPC  # 9216
    U_pad = np.zeros((YT, D), dtype=np.float32)
    U_pad[:Y] = U
    fw_pad = np.zeros((YT, D), dtype=np.float32)
    fw_pad[:Y] = final_w
    fb_pad = np.zeros(YT, dtype=np.float32)
    fb_pad[:Y] = final_b
    tg_pad = np.zeros((B, YT), dtype=np.float32)
    tg_pad[:, :Y] = target
    mask = np.zeros(YT, dtype=np.float32)
    mask[:Y] = 1.0

    in_maps = []
    for i in range(NCORES):
        sl = slice(i * YPC, (i + 1) * YPC)
        in_maps.append({
            "emb": emb_pad,
            "xr": x_dev,
            "cw": cw_dev,
            "cb": cb_dev,
            "ut": np.ascontiguousarray(U_pad[sl].T),
            "fwt": np.ascontiguousarray(fw_pad[sl].T),
            "fbr": np.ascontiguousarray(fb_pad[sl].reshape(NYT, 128).T),
            "mkr": np.ascontiguousarray(mask[sl].reshape(NYT, 128).T),
            "tgt": np.ascontiguousarray(
                tg_pad[:, sl].reshape(B, NYT, 128).transpose(2, 1, 0).reshape(128, NYT * B)
            ),
        })

    nc = _get_nc()
    if RUN_KWARGS.get("trace"):
        _install_ntff_hook()
    res = run_bass_kernel_spmd(nc, in_maps, core_ids=list(range(NCORES)), **RUN_KWARGS)
    LAST_RESULTS[0] = res

    # ---- host-side unshard ----------------------------------------------
    alpha_full = np.empty((B, Y, L), dtype=np.float32)
    yhat_full = np.empty((B, Y), dtype=np.float32)
    loss_sum = 0.0
    for i in range(NCORES):
        out = res.results[i]
        lo = i * YPC
        hi = min((i + 1) * YPC, Y)
        n = hi - lo
        alpha_full[:, lo:hi, :] = out["alpha"][:, :n, :]
        yh = out["yhat"].reshape(128, NYT, B).transpose(2, 1, 0).reshape(B, YPC)
        yhat_full[:, lo:hi] = yh[:, :n]
        loss_sum += float(out["lossp"].sum())
    loss = np.float32(loss_sum / (B * Y))
    return yhat_full, loss, alpha_full


# revision 8
# speedup vs baseline: 1.5900x; 1.5900x over previous
"""ConvAttnPool (CAML-style label-wise attention) Trainium2 kernel.

Strategy: shard the label dim Y across 8 NeuronCores. Y=8921 is padded to
9216 = 8 cores x 1152 labels (9 y-tiles of 128). Each core redundantly
computes embedding+conv+tanh -> h [B, L, D] (cheap), then for its own label
slab computes attention scores, softmax over L, the alpha slab output
[B, 1152, L] (the dominant ~92MB/core HBM write), and its logits/yhat/loss
partials.  No collectives: the host concatenates per-core outputs.

Key device tricks:
  - softmax denominator comes free from the Exp activation's accum_out
  - logits numerator sum_l exp(s)*c uses the fused DVE tensor_tensor_reduce
    (c[y,l] = final_w[y]·h[l] so m = alpha@h is never materialized and no
    transpose of alpha is needed)
  - normalization passes are split between ScalarE and VectorE to balance
    engine load; alpha tiles stream straight to HBM.
"""

import math
import numpy as np

B = 8
L = 2500
E = 100
D = 128
Y = 8921
K = 9
PADH = K // 2  # 4
VOCAB = 50002

NCORES = 8
NYT = 9                    # y-tiles of 128 per core
YPC = NYT * 128            # 1152 labels per core
LC = L // 5                # 500, psum chunk
NT = math.ceil(L / 128)    # 20 token tiles
L_PAD = NT * 128           # 2560

_CACHED_NC = [None]
RUN_KWARGS = {}            # test.py may set dict(trace=True, tmpdir=...)
LAST_RESULTS = [None]      # BassKernelResults from the last run


def _build_nc():
    import concourse.bass as bass
    import concourse.mybir as mybir
    from concourse import bacc
    from concourse.tile import TileContext
    from concourse.masks import make_identity

    f32 = mybir.dt.float32
    f32r = mybir.dt.float32r
    i32 = mybir.dt.int32
    AF = mybir.ActivationFunctionType
    OP = mybir.AluOpType

    nc = bacc.Bacc(None, target_bir_lowering=False)

    emb = nc.dram_tensor("emb", [VOCAB, 128], f32, kind="ExternalInput")
    xr = nc.dram_tensor("xr", [B, 128, NT], i32, kind="ExternalInput")
    cw = nc.dram_tensor("cw", [128, K, 128], f32r, kind="ExternalInput")   # (e,k,d)
    cb = nc.dram_tensor("cb", [128, 1], f32, kind="ExternalInput")
    ut = nc.dram_tensor("ut", [128, YPC], f32r, kind="ExternalInput")      # U^T slab
    fwt = nc.dram_tensor("fwt", [128, YPC], f32r, kind="ExternalInput")    # final_w^T slab
    fbr = nc.dram_tensor("fbr", [128, NYT], f32, kind="ExternalInput")    # final_b
    mkr = nc.dram_tensor("mkr", [128, NYT], f32, kind="ExternalInput")    # validity mask
    tgt = nc.dram_tensor("tgt", [128, NYT * B], f32, kind="ExternalInput")

    alpha = nc.dram_tensor("alpha", [B, YPC, L], f32, kind="ExternalOutput")
    yhat = nc.dram_tensor("yhat", [128, NYT * B], f32, kind="ExternalOutput")
    lossp = nc.dram_tensor("lossp", [128, 1], f32, kind="ExternalOutput")

    with TileContext(nc) as tc:
        with (
            tc.tile_pool(name="const", bufs=1) as const,
            tc.tile_pool(name="ebig", bufs=1) as ebig,
            tc.tile_pool(name="etp", bufs=2) as etp,
            tc.tile_pool(name="hpool", bufs=1) as hpool,
            tc.tile_pool(name="xp", bufs=2) as xp,
            tc.tile_pool(name="expp", bufs=3) as expp,
            tc.tile_pool(name="alp", bufs=3) as alp,
            tc.tile_pool(name="small", bufs=1) as small,
            tc.tile_pool(name="recp", bufs=3) as recp,
            tc.tile_pool(name="trashp", bufs=1) as trashp,
            tc.tile_pool(name="ps_s", bufs=1, space="PSUM") as ps_s,
            tc.tile_pool(name="ps_e", bufs=1, space="PSUM") as ps_e,
            tc.tile_pool(name="ps_c", bufs=2, space="PSUM") as ps_c,
        ):
            identity = const.tile([128, 128], f32)
            make_identity(nc, identity[:])
            ut_sb = const.tile([128, YPC], f32r)
            nc.sync.dma_start(ut_sb[:], ut[:])
            fwt_sb = const.tile([128, YPC], f32r)
            nc.sync.dma_start(fwt_sb[:], fwt[:])
            cw_sb = const.tile([128, K, 128], f32r)
            nc.sync.dma_start(cw_sb[:], cw[:])
            cb_sb = const.tile([128, 1], f32)
            nc.sync.dma_start(cb_sb[:], cb[:])
            fb_sb = const.tile([128, NYT], f32)
            nc.sync.dma_start(fb_sb[:], fbr[:])
            mk_sb = const.tile([128, NYT], f32)
            nc.sync.dma_start(mk_sb[:], mkr[:])
            tgt_sb = const.tile([128, NYT * B], f32)
            nc.sync.dma_start(tgt_sb[:], tgt[:])

            num_all = small.tile([128, NYT * B], f32)
            den_all = small.tile([128, NYT * B], f32)
            hT = hpool.tile([128, B, L], f32r)

            # ---------------- Phase A: embedding + conv + tanh -> hT ------------
            for b in range(B):
                xb = xp.tile([128, NT], i32)
                nc.sync.dma_start(xb[:], xr[b])
                eb = ebig.tile([128, NT, 128], f32)
                for t in range(NT):
                    # NB: a single gather with a [128, NT] offset AP returns
                    # garbage on HW (works in sim) — keep one offset column per op.
                    nc.gpsimd.indirect_dma_start(
                        out=eb[:, t, :],
                        out_offset=None,
                        in_=emb[:],
                        in_offset=bass.IndirectOffsetOnAxis(ap=xb[:, t:t + 1], axis=0),
                    )
                etb = etp.tile([128, 2 * PADH + L], f32r)
                nc.vector.memset(etb[:, 0:PADH].bitcast(f32), 0.0)
                nc.vector.memset(etb[:, PADH + L: 2 * PADH + L].bitcast(f32), 0.0)
                for g in range(math.ceil(NT / 4)):
                    eps = ps_e.tile([128, 512], f32, tag="eps")
                    for j in range(min(4, NT - g * 4)):
                        t = g * 4 + j
                        nc.tensor.transpose(
                            eps[:, j * 128:(j + 1) * 128], eb[:, t, :], identity[:]
                        )
                    ncols = min(512, L - g * 512)
                    nc.scalar.copy(
                        etb[:, PADH + g * 512: PADH + g * 512 + ncols], eps[:, :ncols]
                    )
                sps = ps_s.tile([128, 5, 512], f32, tag="sps")
                for k in range(K):
                    for c in range(5):
                        nc.tensor.matmul(
                            sps[:, c, :LC],
                            lhsT=cw_sb[:, k, :],
                            rhs=etb[:, k + c * LC: k + c * LC + LC],
                            start=(k == 0),
                            stop=(k == K - 1),
                        )
                nc.scalar.activation(
                    hT[:, b, :].rearrange("p (c l) -> p c l", c=5),
                    sps[:, :, :LC],
                    AF.Tanh,
                    bias=cb_sb[:, 0:1],
                )

            # ---------------- Phase B: per (y-tile, batch) ----------------------
            for yt in range(NYT):
                for b in range(B):
                    idx = yt * B + b
                    ysl = slice(yt * 128, (yt + 1) * 128)
                    sps = ps_s.tile([128, 5, 512], f32, tag="sps")
                    for c in range(5):
                        nc.tensor.matmul(
                            sps[:, c, :LC],
                            lhsT=ut_sb[:, ysl],
                            rhs=hT[:, b, c * LC:(c + 1) * LC],
                            start=True,
                            stop=True,
                        )
                    expt = expp.tile([128, L], f32)
                    nc.scalar.activation(
                        expt[:].rearrange("p (c l) -> p c l", c=5),
                        sps[:, :, :LC],
                        AF.Exp,
                        accum_out=den_all[:, idx:idx + 1],
                    )
                    acc5 = recp.tile([128, 5], f32, tag="acc5")
                    for c in range(5):
                        cps = ps_c.tile([128, 512], f32, tag="cps")
                        nc.tensor.matmul(
                            cps[:, :LC],
                            lhsT=fwt_sb[:, ysl],
                            rhs=hT[:, b, c * LC:(c + 1) * LC],
                            start=True,
                            stop=True,
                        )
                        trash = trashp.tile([128, LC], f32, tag="trash")
                        # fused multiply + per-partition sum on DVE.
                        # (tensor_tensor_reduce crashes on HW; op0=bypass breaks
                        # the accumulator — op0=mult with scalar 1.0 is exact.)
                        nc.vector.scalar_tensor_tensor(
                            out=trash[:],
                            in0=expt[:, c * LC:(c + 1) * LC],
                            scalar=1.0,
                            in1=cps[:, :LC],
                            op0=OP.mult,
                            op1=OP.mult,
                            accum_out=acc5[:, c:c + 1],
                        )
                    nc.vector.reduce_sum(
                        num_all[:, idx:idx + 1], acc5[:], axis=mybir.AxisListType.X
                    )
                    rec = recp.tile([128, 1], f32, tag="rec")
                    nc.vector.reciprocal(rec[:], den_all[:, idx:idx + 1])
                    alpt = alp.tile([128, L], f32)
                    if idx % 2 == 0:
                        nc.vector.tensor_scalar_mul(alpt[:], expt[:], rec[:])
                    else:
                        nc.scalar.activation(alpt[:], expt[:], AF.Copy, scale=rec[:, 0:1])
                    nc.sync.dma_start(alpha[b, ysl, :], alpt[:])

            # ---------------- Final: logits, yhat, masked BCE partials ----------
            NB = NYT * B
            recall = small.tile([128, NB], f32)
            nc.vector.reciprocal(recall[:], den_all[:])
            z = small.tile([128, NB], f32)
            nc.vector.tensor_tensor(z[:], num_all[:], recall[:], op=OP.mult)
            z3 = z[:].rearrange("p (t b) -> p t b", t=NYT)
            nc.vector.tensor_tensor(
                z3, z3, fb_sb[:, :, None].to_broadcast([128, NYT, B]), op=OP.add
            )
            yh = small.tile([128, NB], f32)
            nc.scalar.activation(yh[:], z[:], AF.Sigmoid)
            nc.sync.dma_start(yhat[:], yh[:])
            reluz = small.tile([128, NB], f32)
            nc.vector.tensor_scalar_max(reluz[:], z[:], 0.0)
            zt = small.tile([128, NB], f32)
            nc.vector.tensor_tensor(zt[:], z[:], tgt_sb[:], op=OP.mult)
            negabs = small.tile([128, NB], f32)
            nc.vector.scalar_tensor_tensor(
                out=negabs[:], in0=reluz[:], scalar=-2.0, in1=z[:],
                op0=OP.mult, op1=OP.add,
            )
            enb = small.tile([128, NB], f32)
            nc.scalar.activation(enb[:], negabs[:], AF.Exp)
            sp = small.tile([128, NB], f32)
            nc.scalar.activation(sp[:], enb[:], AF.Ln, bias=1.0)
            term = small.tile([128, NB], f32)
            nc.vector.tensor_tensor(term[:], reluz[:], zt[:], op=OP.subtract)
            nc.vector.tensor_tensor(term[:], term[:], sp[:], op=OP.add)
            t3 = term[:].rearrange("p (t b) -> p t b", t=NYT)
            nc.vector.tensor_tensor(
                t3, t3, mk_sb[:, :, None].to_broadcast([128, NYT, B]), op=OP.mult
            )
            lp = small.tile([128, 1], f32)
            nc.vector.reduce_sum(lp[:], term[:], axis=mybir.AxisListType.X)
            nc.sync.dma_start(lossp[:], lp[:])

    nc.finalize()
    return nc


def _get_nc():
    if _CACHED_NC[0] is None:
        _CACHED_NC[0] = _build_nc()
    return _CACHED_NC[0]


def _install_ntff_hook():
    """The RL container's antenv lacks axon_hooks; shim it so trace=True works."""
    import sys, types
    if "antenv.axon_hooks" in sys.modules:
        return
    m = types.ModuleType("antenv.axon_hooks")
    holder = [None]
    m.set_axon_ntff_profile_hook = lambda h: holder.__setitem__(0, h)
    m.get_axon_ntff_profile_hook = lambda: holder[0]
    sys.modules["antenv.axon_hooks"] = m
    try:
        import antenv
        antenv.axon_hooks = m
        from trn_agent_boot.trn_boot import _ntff_profile_via_ctypes
        m.set_axon_ntff_profile_hook(_ntff_profile_via_ctypes("/opt/axon/libaxon_pjrt.so"))
    except Exception:
        pass


def kernel(x, target, emb_table, conv_w, conv_b, U, final_w, final_b):
    from concourse.bass_utils import run_bass_kernel_spmd

    x = np.asarray(x).astype(np.int32)
    target = np.asarray(target, dtype=np.float32)
    emb_table = np.asarray(emb_table, dtype=np.float32)
    conv_w = np.asarray(conv_w, dtype=np.float32)
    conv_b = np.asarray(conv_b, dtype=np.float32)
    U = np.asarray(U, dtype=np.float32)
    final_w = np.asarray(final_w, dtype=np.float32)
    final_b = np.asarray(final_b, dtype=np.float32)

    # ---- host-side shard prep -------------------------------------------
    emb_pad = np.zeros((VOCAB, 128), dtype=np.float32)
    emb_pad[:, :E] = emb_table
    x_pad = np.zeros((B, L_PAD), dtype=np.int32)
    x_pad[:, :L] = x
    x_dev = np.ascontiguousarray(x_pad.reshape(B, NT, 128).transpose(0, 2, 1))
    cw_dev = np.zeros((128, K, 128), dtype=np.float32)
    cw_dev[:E] = conv_w.transpose(1, 2, 0)  # (e,k,d)
    cb_dev = np.ascontiguousarray(conv_b.reshape(128, 1))

    YT = NCORES * YPC  # 9216
    U_pad = np.zeros((YT, D), dtype=np.float32)
    U_pad[:Y] = U
    fw_pad = np.zeros((YT, D), dtype=np.float32)
    fw_pad[:Y] = final_w
    fb_pad = np.zeros(YT, dtype=np.float32)
    fb_pad[:Y] = final_b
    tg_pad = np.zeros((B, YT), dtype=np.float32)
    tg_pad[:, :Y] = target
    mask = np.zeros(YT, dtype=np.float32)
    mask[:Y] = 1.0

    in_maps = []
    for i in range(NCORES):
        sl = slice(i * YPC, (i + 1) * YPC)
        in_maps.append({
            "emb": emb_pad,
            "xr": x_dev,
            "cw": cw_dev,
            "cb": cb_dev,
            "ut": np.ascontiguousarray(U_pad[sl].T),
            "fwt": np.ascontiguousarray(fw_pad[sl].T),
            "fbr": np.ascontiguousarray(fb_pad[sl].reshape(NYT, 128).T),
            "mkr": np.ascontiguousarray(mask[sl].reshape(NYT, 128).T),
            "tgt": np.ascontiguousarray(
                tg_pad[:, sl].reshape(B, NYT, 128).transpose(2, 1, 0).reshape(128, NYT * B)
            ),
        })

    nc = _get_nc()
    if RUN_KWARGS.get("trace"):
        _install_ntff_hook()
    res = run_bass_kernel_spmd(nc, in_maps, core_ids=list(range(NCORES)), **RUN_KWARGS)
    LAST_RESULTS[0] = res

    # ---- host-side unshard ----------------------------------------------
    alpha_full = np.empty((B, Y, L), dtype=np.float32)
    yhat_full = np.empty((B, Y), dtype=np.float32)
    loss_sum = 0.0
    for i in range(NCORES):
        out = res.results[i]
        lo = i * YPC
        hi = min((i + 1) * YPC, Y)
        n = hi - lo
        alpha_full[:, lo:hi, :] = out["alpha"][:, :n, :]
        yh = out["yhat"].reshape(128, NYT, B).transpose(2, 1, 0).reshape(B, YPC)
        yhat_full[:, lo:hi] = yh[:, :n]
        loss_sum += float(out["lossp"].sum())
    loss = np.float32(loss_sum / (B * Y))
    return yhat_full, loss, alpha_full


# revision 9
# speedup vs baseline: 1.8400x; 1.1572x over previous
"""ConvAttnPool (CAML-style label-wise attention) Trainium2 kernel.

Strategy: shard the label dim Y across 8 NeuronCores. Y=8921 is padded to
9216 = 8 cores x 1152 labels (9 y-tiles of 128). Each core redundantly
computes embedding+conv+tanh -> h [B, L, D] (cheap), then for its own label
slab computes attention scores, softmax over L, the alpha slab output
[B, 1152, L] (the dominant ~92MB/core HBM write), and its logits/yhat/loss
partials.  No collectives: the host concatenates per-core outputs.

Key device tricks:
  - softmax denominator comes free from the Exp activation's accum_out
  - logits numerator sum_l exp(s)*c uses the fused DVE tensor_tensor_reduce
    (c[y,l] = final_w[y]·h[l] so m = alpha@h is never materialized and no
    transpose of alpha is needed)
  - normalization passes are split between ScalarE and VectorE to balance
    engine load; alpha tiles stream straight to HBM.
"""

import math
import numpy as np

B = 8
L = 2500
E = 100
D = 128
Y = 8921
K = 9
PADH = K // 2  # 4
VOCAB = 50002

NCORES = 8
NYT = 9                    # y-tiles of 128 per core
YPC = NYT * 128            # 1152 labels per core
LC = L // 5                # 500, psum chunk
NT = math.ceil(L / 128)    # 20 token tiles
L_PAD = NT * 128           # 2560

_CACHED_NC = [None]
RUN_KWARGS = {}            # test.py may set dict(trace=True, tmpdir=...)
LAST_RESULTS = [None]      # BassKernelResults from the last run


def _build_nc():
    import concourse.bass as bass
    import concourse.mybir as mybir
    from concourse import bacc
    from concourse.tile import TileContext
    from concourse.masks import make_identity

    f32 = mybir.dt.float32
    f32r = mybir.dt.float32r
    i32 = mybir.dt.int32
    AF = mybir.ActivationFunctionType
    OP = mybir.AluOpType

    nc = bacc.Bacc(None, target_bir_lowering=False)

    emb = nc.dram_tensor("emb", [VOCAB, 128], f32, kind="ExternalInput")
    xr = nc.dram_tensor("xr", [128, NT], i32, kind="ExternalInput")  # this core's batch
    cw = nc.dram_tensor("cw", [128, K, 128], f32r, kind="ExternalInput")   # (e,k,d)
    cb = nc.dram_tensor("cb", [128, 1], f32, kind="ExternalInput")
    ut = nc.dram_tensor("ut", [128, YPC], f32r, kind="ExternalInput")      # U^T slab
    fwt = nc.dram_tensor("fwt", [128, YPC], f32r, kind="ExternalInput")    # final_w^T slab
    fbr = nc.dram_tensor("fbr", [128, NYT], f32, kind="ExternalInput")    # final_b
    mkr = nc.dram_tensor("mkr", [128, NYT], f32, kind="ExternalInput")    # validity mask
    tgt = nc.dram_tensor("tgt", [128, NYT * B], f32, kind="ExternalInput")

    alpha = nc.dram_tensor("alpha", [B, YPC, L], f32, kind="ExternalOutput")
    yhat = nc.dram_tensor("yhat", [128, NYT * B], f32, kind="ExternalOutput")
    lossp = nc.dram_tensor("lossp", [128, 1], f32, kind="ExternalOutput")

    with TileContext(nc) as tc:
        with (
            tc.tile_pool(name="const", bufs=1) as const,
            tc.tile_pool(name="ebig", bufs=1) as ebig,
            tc.tile_pool(name="etp", bufs=2) as etp,
            tc.tile_pool(name="hpool", bufs=1) as hpool,
            tc.tile_pool(name="xp", bufs=2) as xp,
            tc.tile_pool(name="expp", bufs=3) as expp,
            tc.tile_pool(name="alp", bufs=3) as alp,
            tc.tile_pool(name="small", bufs=1) as small,
            tc.tile_pool(name="recp", bufs=3) as recp,
            tc.tile_pool(name="trashp", bufs=1) as trashp,
            tc.tile_pool(name="dram", bufs=1, space="DRAM") as dram,
            tc.tile_pool(name="ps_s", bufs=1, space="PSUM") as ps_s,
            tc.tile_pool(name="ps_e", bufs=1, space="PSUM") as ps_e,
            tc.tile_pool(name="ps_c", bufs=2, space="PSUM") as ps_c,
        ):
            identity = const.tile([128, 128], f32)
            make_identity(nc, identity[:])
            ut_sb = const.tile([128, YPC], f32r)
            nc.sync.dma_start(ut_sb[:], ut[:])
            fwt_sb = const.tile([128, YPC], f32r)
            nc.sync.dma_start(fwt_sb[:], fwt[:])
            cw_sb = const.tile([128, K, 128], f32r)
            nc.sync.dma_start(cw_sb[:], cw[:])
            cb_sb = const.tile([128, 1], f32)
            nc.sync.dma_start(cb_sb[:], cb[:])
            fb_sb = const.tile([128, NYT], f32)
            nc.sync.dma_start(fb_sb[:], fbr[:])
            mk_sb = const.tile([128, NYT], f32)
            nc.sync.dma_start(mk_sb[:], mkr[:])
            tgt_sb = const.tile([128, NYT * B], f32)
            nc.sync.dma_start(tgt_sb[:], tgt[:])

            num_all = small.tile([128, NYT * B], f32)
            den_all = small.tile([128, NYT * B], f32)
            hT = hpool.tile([128, B, L], f32)

            # ---- Phase A (data-parallel over batch): this core embeds+convs
            # ONLY its own batch, then the 8 cores AllGather h. ---------------
            assert B == NCORES
            xb = xp.tile([128, NT], i32)
            nc.sync.dma_start(xb[:], xr[:])
            eb = ebig.tile([128, NT, 128], f32)
            for t in range(NT):
                # NB: a single gather with a [128, NT] offset AP returns
                # garbage on HW (works in sim) — keep one offset column per op.
                nc.gpsimd.indirect_dma_start(
                    out=eb[:, t, :],
                    out_offset=None,
                    in_=emb[:],
                    in_offset=bass.IndirectOffsetOnAxis(ap=xb[:, t:t + 1], axis=0),
                )
            etb = etp.tile([128, 2 * PADH + L], f32r)
            nc.vector.memset(etb[:, 0:PADH].bitcast(f32), 0.0)
            nc.vector.memset(etb[:, PADH + L: 2 * PADH + L].bitcast(f32), 0.0)
            for g in range(math.ceil(NT / 4)):
                eps = ps_e.tile([128, 512], f32, tag="eps")
                for j in range(min(4, NT - g * 4)):
                    t = g * 4 + j
                    nc.tensor.transpose(
                        eps[:, j * 128:(j + 1) * 128], eb[:, t, :], identity[:]
                    )
                ncols = min(512, L - g * 512)
                nc.scalar.copy(
                    etb[:, PADH + g * 512: PADH + g * 512 + ncols], eps[:, :ncols]
                )
            sps = ps_s.tile([128, 5, 512], f32, tag="sps")
            for k in range(K):
                for c in range(5):
                    nc.tensor.matmul(
                        sps[:, c, :LC],
                        lhsT=cw_sb[:, k, :],
                        rhs=etb[:, k + c * LC: k + c * LC + LC],
                        start=(k == 0),
                        stop=(k == K - 1),
                    )
            h_own = etp.tile([128, L], f32, tag="h_own")
            nc.scalar.activation(
                h_own[:].rearrange("p (c l) -> p c l", c=5),
                sps[:, :, :LC],
                AF.Tanh,
                bias=cb_sb[:, 0:1],
            )
            hin = dram.tile([128, L], f32)
            hout = dram.tile([B * 128, L], f32)
            nc.sync.dma_start(hin[:], h_own[:])
            nc.gpsimd.collective_compute(
                "AllGather",
                mybir.AluOpType.bypass,
                replica_groups=[list(range(NCORES))],
                ins=[hin[:].opt()],
                outs=[hout[:].opt()],
            )
            for b in range(B):
                nc.sync.dma_start(hT[:, b, :], hout[b * 128:(b + 1) * 128, :])

            # ---------------- Phase B: per (y-tile, batch) ----------------------
            for yt in range(NYT):
                for b in range(B):
                    idx = yt * B + b
                    ysl = slice(yt * 128, (yt + 1) * 128)
                    sps = ps_s.tile([128, 5, 512], f32, tag="sps")
                    for c in range(5):
                        nc.tensor.matmul(
                            sps[:, c, :LC],
                            lhsT=ut_sb[:, ysl],
                            rhs=hT[:, b, c * LC:(c + 1) * LC].bitcast(f32r),
                            start=True,
                            stop=True,
                        )
                    expt = expp.tile([128, L], f32)
                    nc.scalar.activation(
                        expt[:].rearrange("p (c l) -> p c l", c=5),
                        sps[:, :, :LC],
                        AF.Exp,
                        accum_out=den_all[:, idx:idx + 1],
                    )
                    acc5 = recp.tile([128, 5], f32, tag="acc5")
                    for c in range(5):
                        cps = ps_c.tile([128, 512], f32, tag="cps")
                        nc.tensor.matmul(
                            cps[:, :LC],
                            lhsT=fwt_sb[:, ysl],
                            rhs=hT[:, b, c * LC:(c + 1) * LC].bitcast(f32r),
                            start=True,
                            stop=True,
                        )
                        trash = trashp.tile([128, LC], f32, tag="trash")
                        # fused multiply + per-partition sum on DVE.
                        # (tensor_tensor_reduce crashes on HW; op0=bypass breaks
                        # the accumulator — op0=mult with scalar 1.0 is exact.)
                        nc.vector.scalar_tensor_tensor(
                            out=trash[:],
                            in0=expt[:, c * LC:(c + 1) * LC],
                            scalar=1.0,
                            in1=cps[:, :LC],
                            op0=OP.mult,
                            op1=OP.mult,
                            accum_out=acc5[:, c:c + 1],
                        )
                    nc.vector.reduce_sum(
                        num_all[:, idx:idx + 1], acc5[:], axis=mybir.AxisListType.X
                    )
                    rec = recp.tile([128, 1], f32, tag="rec")
                    nc.vector.reciprocal(rec[:], den_all[:, idx:idx + 1])
                    alpt = alp.tile([128, L], f32)
                    if idx % 2 == 0:
                        nc.vector.tensor_scalar_mul(alpt[:], expt[:], rec[:])
                    else:
                        nc.scalar.activation(alpt[:], expt[:], AF.Copy, scale=rec[:, 0:1])
                    nc.sync.dma_start(alpha[b, ysl, :], alpt[:])

            # ---------------- Final: logits, yhat, masked BCE partials ----------
            NB = NYT * B
            recall = small.tile([128, NB], f32)
            nc.vector.reciprocal(recall[:], den_all[:])
            z = small.tile([128, NB], f32)
            nc.vector.tensor_tensor(z[:], num_all[:], recall[:], op=OP.mult)
            z3 = z[:].rearrange("p (t b) -> p t b", t=NYT)
            nc.vector.tensor_tensor(
                z3, z3, fb_sb[:, :, None].to_broadcast([128, NYT, B]), op=OP.add
            )
            yh = small.tile([128, NB], f32)
            nc.scalar.activation(yh[:], z[:], AF.Sigmoid)
            nc.sync.dma_start(yhat[:], yh[:])
            reluz = small.tile([128, NB], f32)
            nc.vector.tensor_scalar_max(reluz[:], z[:], 0.0)
            zt = small.tile([128, NB], f32)
            nc.vector.tensor_tensor(zt[:], z[:], tgt_sb[:], op=OP.mult)
            negabs = small.tile([128, NB], f32)
            nc.vector.scalar_tensor_tensor(
                out=negabs[:], in0=reluz[:], scalar=-2.0, in1=z[:],
                op0=OP.mult, op1=OP.add,
            )
            enb = small.tile([128, NB], f32)
            nc.scalar.activation(enb[:], negabs[:], AF.Exp)
            sp = small.tile([128, NB], f32)
            nc.scalar.activation(sp[:], enb[:], AF.Ln, bias=1.0)
            term = small.tile([128, NB], f32)
            nc.vector.tensor_tensor(term[:], reluz[:], zt[:], op=OP.subtract)
            nc.vector.tensor_tensor(term[:], term[:], sp[:], op=OP.add)
            t3 = term[:].rearrange("p (t b) -> p t b", t=NYT)
            nc.vector.tensor_tensor(
                t3, t3, mk_sb[:, :, None].to_broadcast([128, NYT, B]), op=OP.mult
            )
            lp = small.tile([128, 1], f32)
            nc.vector.reduce_sum(lp[:], term[:], axis=mybir.AxisListType.X)
            nc.sync.dma_start(lossp[:], lp[:])

    nc.finalize()
    return nc


def _get_nc():
    if _CACHED_NC[0] is None:
        _CACHED_NC[0] = _build_nc()
    return _CACHED_NC[0]


def _install_ntff_hook():
    """The RL container's antenv lacks axon_hooks; shim it so trace=True works."""
    import sys, types
    if "antenv.axon_hooks" in sys.modules:
        return
    m = types.ModuleType("antenv.axon_hooks")
    holder = [None]
    m.set_axon_ntff_profile_hook = lambda h: holder.__setitem__(0, h)
    m.get_axon_ntff_profile_hook = lambda: holder[0]
    sys.modules["antenv.axon_hooks"] = m
    try:
        import antenv
        antenv.axon_hooks = m
        from trn_agent_boot.trn_boot import _ntff_profile_via_ctypes
        m.set_axon_ntff_profile_hook(_ntff_profile_via_ctypes("/opt/axon/libaxon_pjrt.so"))
    except Exception:
        pass


def kernel(x, target, emb_table, conv_w, conv_b, U, final_w, final_b):
    from concourse.bass_utils import run_bass_kernel_spmd

    x = np.asarray(x).astype(np.int32)
    target = np.asarray(target, dtype=np.float32)
    emb_table = np.asarray(emb_table, dtype=np.float32)
    conv_w = np.asarray(conv_w, dtype=np.float32)
    conv_b = np.asarray(conv_b, dtype=np.float32)
    U = np.asarray(U, dtype=np.float32)
    final_w = np.asarray(final_w, dtype=np.float32)
    final_b = np.asarray(final_b, dtype=np.float32)

    # ---- host-side shard prep -------------------------------------------
    emb_pad = np.zeros((VOCAB, 128), dtype=np.float32)
    emb_pad[:, :E] = emb_table
    x_pad = np.zeros((B, L_PAD), dtype=np.int32)
    x_pad[:, :L] = x
    x_dev = np.ascontiguousarray(x_pad.reshape(B, NT, 128).transpose(0, 2, 1))
    cw_dev = np.zeros((128, K, 128), dtype=np.float32)
    cw_dev[:E] = conv_w.transpose(1, 2, 0)  # (e,k,d)
    cb_dev = np.ascontiguousarray(conv_b.reshape(128, 1))

    YT = NCORES * YPC  # 9216
    U_pad = np.zeros((YT, D), dtype=np.float32)
    U_pad[:Y] = U
    fw_pad = np.zeros((YT, D), dtype=np.float32)
    fw_pad[:Y] = final_w
    fb_pad = np.zeros(YT, dtype=np.float32)
    fb_pad[:Y] = final_b
    tg_pad = np.zeros((B, YT), dtype=np.float32)
    tg_pad[:, :Y] = target
    mask = np.zeros(YT, dtype=np.float32)
    mask[:Y] = 1.0

    in_maps = []
    for i in range(NCORES):
        sl = slice(i * YPC, (i + 1) * YPC)
        in_maps.append({
            "emb": emb_pad,
            "xr": np.ascontiguousarray(x_dev[i]),
            "cw": cw_dev,
            "cb": cb_dev,
            "ut": np.ascontiguousarray(U_pad[sl].T),
            "fwt": np.ascontiguousarray(fw_pad[sl].T),
            "fbr": np.ascontiguousarray(fb_pad[sl].reshape(NYT, 128).T),
            "mkr": np.ascontiguousarray(mask[sl].reshape(NYT, 128).T),
            "tgt": np.ascontiguousarray(
                tg_pad[:, sl].reshape(B, NYT, 128).transpose(2, 1, 0).reshape(128, NYT * B)
            ),
        })

    nc = _get_nc()
    if RUN_KWARGS.get("trace"):
        _install_ntff_hook()
    res = run_bass_kernel_spmd(nc, in_maps, core_ids=list(range(NCORES)), **RUN_KWARGS)
    LAST_RESULTS[0] = res

    # ---- host-side unshard ----------------------------------------------
    alpha_full = np.empty((B, Y, L), dtype=np.float32)
    yhat_full = np.empty((B, Y), dtype=np.float32)
    loss_sum = 0.0
    for i in range(NCORES):
        out = res.results[i]
        lo = i * YPC
        hi = min((i + 1) * YPC, Y)
        n = hi - lo
        alpha_full[:, lo:hi, :] = out["alpha"][:, :n, :]
        yh = out["yhat"].reshape(128, NYT, B).transpose(2, 1, 0).reshape(B, YPC)
        yhat_full[:, lo:hi] = yh[:, :n]
        loss_sum += float(out["lossp"].sum())
    loss = np.float32(loss_sum / (B * Y))
    return yhat_full, loss, alpha_full
